# revision 8
# baseline (speedup 1.0000x reference)
"""Trainium2 Bass kernel: 2-layer LSTM language-model loss (fp8 DoubleRow).

Reference: x = embedding[features]; 2-layer LSTM over T=64 steps with
sequence-length state freezing; logits = out @ softmax_w + softmax_b;
masked mean cross-entropy -> scalar.

Strategy (8 NeuronCores, SPMD, zero cross-core collectives):
  * Every core runs the identical full-batch (B=128) recurrence; the
    large projection is sharded over the vocab dim (core c owns columns
    [c*1250, (c+1)*1250)), producing per (b,t) S_c = sum_v exp(logit_v).
    The label logit is a row-wise dot with a host-pregathered row of
    softmax_w^T, gated to core 0.  Host: xent = log(sum_c S_c) - LD.
  * All matmuls run fp8e4m3 + MatmulPerfMode.DoubleRow (K virtualized
    to 256 -> ~1.8x the bf16 streaming rate).  Scales: stationaries
    (x^T, o^T, h^T) carry SH=8x, weights SW=32x (with the sigmoid-trick
    0.5 folded into f/i/o weight columns), undone by ACT scale 1/256.
  * Host pre-gathers/transposes the embedded inputs (xt8) and the label
    rows of softmax_w^T (wl8) -- both are pure data layout, no FLOPs --
    so the loop has no indirect DMA and no PE transposes for x.
  * The h state lives ONLY in transposed (stationary) layout:
    hT_new = (1-m)*hT_old + oT, with (1-m) broadcast along partitions
    from a host-shipped slen row (slbc).  No h transposes.
  * Emission order keeps the PE fed with ready work (next step's x-part
    gates, the previous step's projection) while the serial cell chain
    runs on ACT/DVE, so the HAM clock gate stays at 2.4 GHz.

Assumes b0 = b1 = softmax_b = 0 (verified at runtime).
"""

import numpy as np
import ml_dtypes


def _ensure_path():
    try:
        import concourse  # noqa: F401
    except ImportError:
        import sys

        for p in ("/opt/trn_rl_repo", "/root/.axon_site/_ro/trn_rl_repo"):
            if p not in sys.path:
                sys.path.append(p)


_ensure_path()

from contextlib import ExitStack  # noqa: E402

import concourse.bass as bass  # noqa: E402
import concourse.bacc as bacc  # noqa: E402
import concourse.tile as tile  # noqa: E402
from concourse import mybir  # noqa: E402
from concourse.alu_op_type import AluOpType as OP  # noqa: E402
from concourse.bass_utils import run_bass_kernel_spmd  # noqa: E402
from concourse.masks import make_identity  # noqa: E402

dt = mybir.dt
AF = mybir.ActivationFunctionType
DR = mybir.MatmulPerfMode.DoubleRow

import os as _os

B = 128
T = int(_os.environ.get("KERNEL_T_OVERRIDE", "64"))
H = 512
V = 10000
NCORES = 8
VSH = V // NCORES  # 1250
G = 4 * H  # 2048
KH = H // 128  # 4 k-chunks per 512-wide contraction
MM_DT = dt.bfloat16
F8 = dt.float8e4
NP_F8 = ml_dtypes.float8_e4m3
NP_BF16 = ml_dtypes.bfloat16
SW = 32.0   # weight scale into fp8
SH = 8.0    # stationary-activation scale into fp8
GS = 1.0 / (SW * SH)  # activation un-scale
# projection free-dim chunks (PSUM bank = 512 fp32)
PCHUNKS = [(0, 512), (512, 1024), (1024, VSH)]

_CACHE: dict = {}


def _dr2(ap512, d):
    """[128, 512] T-layout slice for DoubleRow chunk d: [128, 2, 128]."""
    return ap512[:, 256 * d:256 * (d + 1)].rearrange("p (two m) -> p two m",
                                                     two=2)


def _emit(nc, tc, ext):
    f32 = dt.float32
    with ExitStack() as ctx:
        cpool = ctx.enter_context(tc.tile_pool(name="const", bufs=1))
        state = ctx.enter_context(tc.tile_pool(name="state", bufs=2))
        wp = ctx.enter_context(tc.tile_pool(name="work", bufs=3))
        gpsum = ctx.enter_context(tc.tile_pool(name="gpsum", bufs=2, space="PSUM"))
        tpsum = ctx.enter_context(tc.tile_pool(name="tpsum", bufs=1, space="PSUM"))
        ppsum = ctx.enter_context(tc.tile_pool(name="ppsum", bufs=1, space="PSUM"))

        # ---- constants / inputs -------------------------------------------
        ldsc = cpool.tile([B, 1], f32)
        nc.sync.dma_start(ldsc[:], ext["ldsc"][:, :])
        Mh = cpool.tile([B, T], f32)   # 0.5*m
        nc.sync.dma_start(Mh[:], ext["Mh"][:, :])
        M1h = cpool.tile([B, T], f32)  # 1-0.5*m
        nc.sync.dma_start(M1h[:], ext["M1h"][:, :])
        M4 = cpool.tile([B, T], f32)   # 4*m
        nc.sync.dma_start(M4[:], ext["M4"][:, :])
        slbc = cpool.tile([128, H], MM_DT)  # slen[b] bcast along p, 4x tiled
        nc.sync.dma_start(slbc[:], ext["slbc"][:, :])

        # per-k-chunk DMAs: first gate matmuls start before the full set lands
        w0 = cpool.tile([128, 2 * KH, G], F8)
        for k in range(2 * KH):
            nc.sync.dma_start(w0[:, k, :], ext["w0"][k, :, :])
        w1 = cpool.tile([128, 2 * KH, G], F8)
        for k in range(2 * KH):
            nc.sync.dma_start(w1[:, k, :], ext["w1"][k, :, :])
        wsm = cpool.tile([128, KH, VSH], F8)
        nc.sync.dma_start(wsm[:], ext["wsm"][:, :, :].rearrange("k p n -> p k n"))
        # pre-transposed, pre-scaled embedded inputs and label rows
        tch = min(8, T)
        xt_all = cpool.tile([128, T, KH, B], F8)
        for tt_ in range(0, T, tch):
            nc.sync.dma_start(xt_all[:, tt_:tt_ + tch, :, :],
                              ext["xt8"][:, tt_:tt_ + tch, :, :])
        wl_all = cpool.tile([128, T, H], F8)
        for tt_ in range(0, T, tch):
            nc.sync.dma_start(wl_all[:, tt_:tt_ + tch, :],
                              ext["wl8"][:, tt_:tt_ + tch, :])

        identB = cpool.tile([128, 128], MM_DT)
        make_identity(nc, identB[:])

        Sacc = cpool.tile([B, T], f32)
        LDacc = cpool.tile([B, T], f32)

        # ---- initial states ------------------------------------------------
        c_st = {}
        hT_st = {}
        for li in (0, 1):
            c_st[li] = state.tile([B, H], f32, name=f"c{li}", tag=f"c{li}")
            nc.vector.memset(c_st[li][:], 0.0)
            hT_st[li] = state.tile([128, H], F8, name=f"hT{li}", tag=f"hT{li}")
            nc.vector.memset(hT_st[li][:], 0.0)

        def alloc_gates(li):
            # layer-0 pairs double-buffer across steps (x-part of t+1 runs
            # while t is consumed); layer-1 needs only one live pair.
            return [gpsum.tile([B, G // 2], f32, name=f"g{li}", tag=f"g{li}",
                               bufs=(2 if li == 0 else 1))
                    for _ in (0, 1)]

        def gates_part(halves, lhs_fn, w_tile, part, start, stop):
            # half-outer: each [B,1024] PSUM half finishes its accumulation
            # early so the cell's ACT read starts sooner.  DoubleRow: each
            # (half, n) slice accumulates K=512 as two K=256 chunks.
            k0 = 0 if part == "x" else KH
            for half in (0, 1):
                gh = halves[half]
                for d in (0, 1):
                    lhs = lhs_fn(d)
                    for n in (0, 1):
                        sl = slice(512 * n, 512 * (n + 1))
                        wsl = slice(1024 * half + 512 * n,
                                    1024 * half + 512 * (n + 1))
                        nc.tensor.matmul(gh[:, sl], lhs,
                                         w_tile[:, k0 + 2 * d:k0 + 2 * d + 2, wsl],
                                         start=(start and d == 0),
                                         stop=(stop and d == 1),
                                         perf_mode=DR)

        def cell(t, li, ghalves):
            """LSTM cell elementwise chain. gates ordered [f, i | o, cg].

            All gate columns need the same Tanh(g/256) (sigmoid 0.5 factors
            are folded into the f/i/o weight columns), so each PSUM half is
            one ACT op.  Returns o8 = 8*m*h_new (bf16, B-layout)."""
            gA, gB = ghalves
            mht = Mh[:, t:t + 1]
            m1ht = M1h[:, t:t + 1]
            m4t = M4[:, t:t + 1]

            thA = wp.tile([B, G // 2], MM_DT, name="thA", tag="thA")
            nc.scalar.activation(thA[:], gA[:], AF.Tanh, scale=GS)
            thB = wp.tile([B, G // 2], MM_DT, name="thB", tag="thB")
            nc.scalar.activation(thB[:], gB[:], AF.Tanh, scale=GS)

            fp = wp.tile([B, H], MM_DT, name="fp", tag="fp")
            nc.vector.tensor_scalar(out=fp[:], in0=thA[:, 0:512], scalar1=mht,
                                    scalar2=m1ht, op0=OP.mult, op1=OP.add)
            ip = wp.tile([B, H], MM_DT, name="ip", tag="ip")
            nc.vector.tensor_scalar(out=ip[:], in0=thA[:, 512:1024], scalar1=mht,
                                    scalar2=mht, op0=OP.mult, op1=OP.add)
            osm = wp.tile([B, H], MM_DT, name="osm", tag="osm")
            nc.vector.tensor_scalar(out=osm[:], in0=thB[:, 0:512], scalar1=m4t,
                                    scalar2=m4t, op0=OP.mult, op1=OP.add)

            r = wp.tile([B, H], f32, name="r", tag="r")
            nc.vector.tensor_tensor(out=r[:], in0=fp[:], in1=c_st[li][:],
                                    op=OP.mult)
            q = wp.tile([B, H], MM_DT, name="q", tag="q")
            nc.vector.tensor_tensor(out=q[:], in0=ip[:], in1=thB[:, 512:1024],
                                    op=OP.mult)
            c_new = state.tile([B, H], f32, name=f"c{li}", tag=f"c{li}")
            nc.vector.tensor_tensor(out=c_new[:], in0=r[:], in1=q[:], op=OP.add)
            c_st[li] = c_new
            tc_ = wp.tile([B, H], MM_DT, name="tc_", tag="tc_")
            nc.scalar.activation(tc_[:], c_new[:], AF.Tanh)
            o8 = wp.tile([B, H], MM_DT, name=f"o{li}", tag=f"o{li}")
            nc.vector.tensor_tensor(out=o8[:], in0=osm[:], in1=tc_[:],
                                    op=OP.mult)
            return o8

        def transpose_o(o8, li):
            # o0T is on the critical path (feeds layer-1 x gates same step):
            # PE transpose.  o1T is consumed only by next step's projection:
            # DMA xbar transpose (off the PE).
            oTb = wp.tile([128, H], MM_DT, name=f"oTb{li}", tag=f"oTb{li}")
            if li == 0:
                ps = tpsum.tile([128, H], MM_DT, name="tp", tag="tp")
                for kc in range(KH):
                    sl = slice(128 * kc, 128 * (kc + 1))
                    nc.tensor.transpose(ps[:, sl], o8[:, sl], identB[:])
                oT = wp.tile([128, H], F8, name=f"oT{li}", tag=f"oT{li}")
                nc.scalar.copy(oT[:, 0:256], ps[:, 0:256])
                nc.vector.tensor_copy(oT[:, 256:512], ps[:, 256:512])
                return oT
            for kc in range(KH):
                sl = slice(128 * kc, 128 * (kc + 1))
                nc.sync.dma_start_transpose(oTb[:, sl], o8[:, sl])
            oT = wp.tile([128, H], F8, name=f"oT{li}", tag=f"oT{li}")
            nc.scalar.copy(oT[:], oTb[:])
            return oT

        def update_hT(li, oT, m1bc):
            # hT_new = (1-m)*hT_old + oT (exact freeze for finished rows);
            # runs on the otherwise-idle GPSIMD so DVE stays on the cell chain
            tmp = wp.tile([128, H], MM_DT, name="htmp", tag="htmp")
            nc.gpsimd.tensor_tensor(out=tmp[:], in0=hT_st[li][:], in1=m1bc[:],
                                    op=OP.mult)
            hT = state.tile([128, H], F8, name=f"hT{li}", tag=f"hT{li}")
            nc.gpsimd.tensor_tensor(out=hT[:], in0=tmp[:], in1=oT[:], op=OP.add)
            hT_st[li] = hT

        def project(t, o1T8, o1_8):
            sps = []
            for (n0, n1) in PCHUNKS:
                w = n1 - n0
                pp = ppsum.tile([128, 512], f32, name="pp", tag="pp")
                for d in (0, 1):
                    nc.tensor.matmul(pp[:, 0:w], _dr2(o1T8[:], d),
                                     wsm[:, 2 * d:2 * d + 2, n0:n1],
                                     start=(d == 0), stop=(d == 1),
                                     perf_mode=DR)
                sp_i = wp.tile([B, 1], f32, name="sp_i", tag="sp_i")
                exp_scr = wp.tile([B, 512], MM_DT, name="exp_scr", tag="exp_scr")
                nc.scalar.activation(exp_scr[:, 0:w], pp[:, 0:w], AF.Exp,
                                     scale=GS, accum_out=sp_i[:])
                sps.append(sp_i)
            s01 = wp.tile([B, 1], f32, name="s01", tag="s01")
            nc.vector.tensor_tensor(out=s01[:], in0=sps[0][:], in1=sps[1][:],
                                    op=OP.add)
            nc.vector.tensor_tensor(out=Sacc[:, t:t + 1], in0=s01[:],
                                    in1=sps[2][:], op=OP.add)
            ld_scr = wp.tile([B, H], MM_DT, name="ld_scr", tag="ld_scr")
            nc.vector.scalar_tensor_tensor(out=ld_scr[:], in0=o1_8[:],
                                           scalar=GS, in1=wl_all[:, t, :],
                                           op0=OP.mult, op1=OP.mult,
                                           accum_out=LDacc[:, t:t + 1])

        # ---- software-pipelined main loop ---------------------------------
        # Emission order = per-engine issue order.  Keep ready MM work (next
        # step's x-part, previous step's projection) queued on the PE while
        # the serial cell chains run on ACT/DVE, so the PE never idles past
        # the HAM re-throttle window.
        def xt_lhs(t):
            return lambda d: xt_all[:, t, 2 * d:2 * d + 2, :]

        g0 = alloc_gates(0)
        gates_part(g0, xt_lhs(0), w0, "x", start=True, stop=True)  # t=0: no rec
        o1_prev = None
        o1T_prev = None
        for t in range(T):
            # (1-m[t]) broadcast along partitions, tiled 4x: for hT updates
            if t + 1 < T:
                m1bc = wp.tile([128, H], MM_DT, name="m1bc", tag="m1bc")
                nc.gpsimd.tensor_scalar(out=m1bc[:], in0=slbc[:],
                                        scalar1=float(t), scalar2=None,
                                        op0=OP.is_le)
            g1 = None
            if t > 0:
                g1 = alloc_gates(1)
                gates_part(g1, lambda d: _dr2(hT_st[1][:], d), w1, "h",
                           start=True, stop=False)
            o0 = cell(t, 0, g0)
            if t + 1 < T:
                g0 = alloc_gates(0)
                gates_part(g0, xt_lhs(t + 1), w0, "x", start=True, stop=False)
            o0T = transpose_o(o0, 0)
            if g1 is None:
                g1 = alloc_gates(1)
                gates_part(g1, lambda d: _dr2(o0T[:], d), w1, "x",
                           start=True, stop=True)
            else:
                gates_part(g1, lambda d: _dr2(o0T[:], d), w1, "x",
                           start=False, stop=True)
            if t + 1 < T:
                update_hT(0, o0T, m1bc)
            if o1T_prev is not None:
                project(t - 1, o1T_prev, o1_prev)
            o1 = cell(t, 1, g1)
            if t + 1 < T:
                gates_part(g0, lambda d: _dr2(hT_st[0][:], d), w0, "h",
                           start=False, stop=True)
            o1T = transpose_o(o1, 1)
            if t + 1 < T:
                update_hT(1, o1T, m1bc)
            o1_prev, o1T_prev = o1, o1T
        project(T - 1, o1T_prev, o1_prev)

        # gate LD to core 0 (ldsc = 1 there, 0 elsewhere)
        nc.vector.tensor_scalar(out=LDacc[:], in0=LDacc[:],
                                scalar1=ldsc[:, 0:1], scalar2=None,
                                op0=OP.mult)
        nc.sync.dma_start(ext["S"][:, :], Sacc[:])
        nc.sync.dma_start(ext["LD"][:, :], LDacc[:])


def _build():
    if "nc" in _CACHE:
        return _CACHE["nc"]
    nc = bacc.Bacc("TRN2", target_bir_lowering=False, debug=False,
                   num_devices=NCORES)
    ext = {
        "ldsc": nc.declare_dram_parameter("ldsc", [B, 1], dt.float32,
                                          isOutput=False),
        "Mh": nc.declare_dram_parameter("Mh", [B, T], dt.float32,
                                        isOutput=False),
        "M1h": nc.declare_dram_parameter("M1h", [B, T], dt.float32,
                                         isOutput=False),
        "M4": nc.declare_dram_parameter("M4", [B, T], dt.float32,
                                        isOutput=False),
        "slbc": nc.declare_dram_parameter("slbc", [128, H], MM_DT,
                                          isOutput=False),
        "w0": nc.declare_dram_parameter("w0", [2 * KH, 128, G], F8,
                                        isOutput=False),
        "w1": nc.declare_dram_parameter("w1", [2 * KH, 128, G], F8,
                                        isOutput=False),
        "wsm": nc.declare_dram_parameter("wsm", [KH, 128, VSH], F8,
                                         isOutput=False),
        "xt8": nc.declare_dram_parameter("xt8", [128, T, KH, B], F8,
                                         isOutput=False),
        "wl8": nc.declare_dram_parameter("wl8", [128, T, H], F8,
                                         isOutput=False),
        "S": nc.declare_dram_parameter("S", [B, T], dt.float32, isOutput=True),
        "LD": nc.declare_dram_parameter("LD", [B, T], dt.float32, isOutput=True),
    }
    with tile.TileContext(nc) as tc:
        _emit(nc, tc, ext)
    nc.compile()
    _CACHE["nc"] = nc
    return nc


def _pack_w(Wx, Wh):
    w = np.concatenate([np.asarray(Wx, np.float32), np.asarray(Wh, np.float32)],
                       axis=0)  # [2H, 4H] rows: x-part then h-part
    # reorder gate blocks [i, cg, f, o] -> [f, i, o, cg]
    w = np.concatenate([w[:, 1024:1536], w[:, 0:512], w[:, 1536:2048],
                        w[:, 512:1024]], axis=1)
    # sigmoid trick: tanh(0.5*g) for f/i/o -> fold 0.5 into those columns
    w[:, 0:1536] *= 0.5
    w *= SW
    return np.ascontiguousarray(w.reshape(2 * KH, 128, G)).astype(NP_F8)


def kernel(features, labels, seq_lengths, seq_mask, embedding,
           W0x, W0h, b0, W1x, W1h, b1, softmax_w, softmax_b,
           _trace_dir=None):
    for name, b in (("b0", b0), ("b1", b1), ("softmax_b", softmax_b)):
        if np.any(np.asarray(b, np.float32) != 0.0):
            raise NotImplementedError(f"{name} != 0 not supported")

    feats = np.asarray(features, np.int64)[:, :T]
    labs = np.asarray(labels, np.int64)[:, :T]
    slen = np.asarray(seq_lengths, np.int32).astype(np.float32)  # [B]
    mask = np.asarray(seq_mask, np.float32)[:, :T]
    emb = np.asarray(embedding, np.float32)
    w0 = _pack_w(W0x, W0h)
    w1 = _pack_w(W1x, W1h)
    wsm_f = np.asarray(softmax_w, np.float32) * SW
    wsm_r = wsm_f.reshape(KH, 128, V)

    # host-side data prep (layout only, no model math):
    # xt8[p, t, kc, b] = SH * emb[features[b, t], kc*128 + p]
    x = emb[feats] * SH                              # [B, T, H]
    xt8 = np.ascontiguousarray(
        x.reshape(B, T, KH, 128).transpose(3, 1, 2, 0)).astype(NP_F8)
    # wl8[b, t, :] = SW * softmax_w[:, labels[b, t]]
    wl8 = np.ascontiguousarray(
        wsm_f.T[labs].transpose(0, 1, 2)).astype(NP_F8)  # [B, T, H]
    # masks
    m = (np.arange(T)[None, :] < slen[:, None]).astype(np.float32)  # [B, T]
    Mh = 0.5 * m
    M1h = 1.0 - 0.5 * m
    M4 = 4.0 * m
    # slbc[p, kc*128+b] = slen[b]
    slbc = np.ascontiguousarray(
        np.tile(slen[None, None, :], (128, KH, 1)).reshape(128, H)
    ).astype(NP_BF16)

    nc = _build()
    in_maps = []
    for c in range(NCORES):
        in_maps.append({
            "ldsc": np.full((B, 1), 1.0 if c == 0 else 0.0, np.float32),
            "Mh": Mh, "M1h": M1h, "M4": M4, "slbc": slbc,
            "w0": w0,
            "w1": w1,
            "wsm": np.ascontiguousarray(
                wsm_r[:, :, c * VSH:(c + 1) * VSH]).astype(NP_F8),
            "xt8": xt8,
            "wl8": wl8,
        })

    kwargs = {}
    if _trace_dir is not None:
        kwargs = dict(trace=True, tmpdir=_trace_dir)
    res = run_bass_kernel_spmd(nc, in_maps, list(range(NCORES)), **kwargs)
    _CACHE["last_results"] = res

    S = np.zeros((B, T), np.float64)
    LD = np.zeros((B, T), np.float64)
    for c in range(NCORES):
        S += np.asarray(res.results[c]["S"], np.float64)
        LD += np.asarray(res.results[c]["LD"], np.float64)

    xent = np.log(S) - LD
    loss_t = (xent * mask).sum(axis=0) / (mask.sum(axis=0) + 1e-12)
    cost = loss_t.mean()
    return np.asarray(cost, np.float32)


# revision 9
# speedup vs baseline: 1.7228x; 1.7228x over previous
"""Trainium2 Bass kernel: 2-layer LSTM language-model loss (fp8 DoubleRow).

Reference: x = embedding[features]; 2-layer LSTM over T=64 steps with
sequence-length state freezing; logits = out @ softmax_w + softmax_b;
masked mean cross-entropy -> scalar.

Strategy (8 NeuronCores, SPMD, zero cross-core collectives):
  * Every core runs the identical full-batch (B=128) recurrence; the
    large projection is sharded over the vocab dim (core c owns columns
    [c*1250, (c+1)*1250)), producing per (b,t) S_c = sum_v exp(logit_v).
    The label logit is a row-wise dot with a host-pregathered row of
    softmax_w^T, gated to core 0.  Host: xent = log(sum_c S_c) - LD.
  * All matmuls run fp8e4m3 + MatmulPerfMode.DoubleRow (K virtualized
    to 256 -> ~1.8x the bf16 streaming rate).  Scales: stationaries
    (x^T, o^T, h^T) carry SH=8x, weights SW=32x (with the sigmoid-trick
    0.5 folded into f/i/o weight columns), undone by ACT scale 1/256.
  * Host pre-gathers/transposes the embedded inputs (xt8) and the label
    rows of softmax_w^T (wl8) -- both are pure data layout, no FLOPs --
    so the loop has no indirect DMA and no PE transposes for x.
  * The h state lives ONLY in transposed (stationary) layout:
    hT_new = (1-m)*hT_old + oT, with (1-m) broadcast along partitions
    from a host-shipped slen row (slbc).  No h transposes.
  * Emission order keeps the PE fed with ready work (next step's x-part
    gates, the previous step's projection) while the serial cell chain
    runs on ACT/DVE, so the HAM clock gate stays at 2.4 GHz.

Assumes b0 = b1 = softmax_b = 0 (verified at runtime).
"""

import numpy as np
import ml_dtypes


def _ensure_path():
    try:
        import concourse  # noqa: F401
    except ImportError:
        import sys

        for p in ("/opt/trn_rl_repo", "/root/.axon_site/_ro/trn_rl_repo"):
            if p not in sys.path:
                sys.path.append(p)


_ensure_path()

from contextlib import ExitStack  # noqa: E402

import concourse.bass as bass  # noqa: E402
import concourse.bacc as bacc  # noqa: E402
import concourse.tile as tile  # noqa: E402
from concourse import mybir  # noqa: E402
from concourse.alu_op_type import AluOpType as OP  # noqa: E402
from concourse.bass_utils import run_bass_kernel_spmd  # noqa: E402
from concourse.masks import make_identity  # noqa: E402

dt = mybir.dt
AF = mybir.ActivationFunctionType
DR = mybir.MatmulPerfMode.DoubleRow

import os as _os

B = 128
T = int(_os.environ.get("KERNEL_T_OVERRIDE", "64"))
H = 512
V = 10000
NCORES = 8
VSH = V // NCORES  # 1250
G = 4 * H  # 2048
KH = H // 128  # 4 k-chunks per 512-wide contraction
MM_DT = dt.bfloat16
F8 = dt.float8e4
NP_F8 = ml_dtypes.float8_e4m3
NP_BF16 = ml_dtypes.bfloat16
SW = 32.0   # weight scale into fp8
SH = 8.0    # stationary-activation scale into fp8
GS = 1.0 / (SW * SH)  # activation un-scale
# projection free-dim chunks (PSUM bank = 512 fp32)
PCHUNKS = [(0, 512), (512, 1024), (1024, VSH)]

_CACHE: dict = {}


def _dr2(ap512, d):
    """[128, 512] T-layout slice for DoubleRow chunk d: [128, 2, 128]."""
    return ap512[:, 256 * d:256 * (d + 1)].rearrange("p (two m) -> p two m",
                                                     two=2)


def _emit(nc, tc, ext):
    f32 = dt.float32
    with ExitStack() as ctx:
        cpool = ctx.enter_context(tc.tile_pool(name="const", bufs=1))
        state = ctx.enter_context(tc.tile_pool(name="state", bufs=2))
        wp = ctx.enter_context(tc.tile_pool(name="work", bufs=3))
        gpsum = ctx.enter_context(tc.tile_pool(name="gpsum", bufs=2, space="PSUM"))
        tpsum = ctx.enter_context(tc.tile_pool(name="tpsum", bufs=1, space="PSUM"))
        ppsum = ctx.enter_context(tc.tile_pool(name="ppsum", bufs=1, space="PSUM"))

        # ---- constants / inputs -------------------------------------------
        ldsc = cpool.tile([B, 1], f32)
        nc.sync.dma_start(ldsc[:], ext["ldsc"][:, :])
        Mh = cpool.tile([B, T], f32)   # 0.5*m
        nc.sync.dma_start(Mh[:], ext["Mh"][:, :])
        M1h = cpool.tile([B, T], f32)  # 1-0.5*m
        nc.sync.dma_start(M1h[:], ext["M1h"][:, :])
        M4 = cpool.tile([B, T], f32)   # 4*m
        nc.sync.dma_start(M4[:], ext["M4"][:, :])
        slbc = cpool.tile([128, H], MM_DT)  # slen[b] bcast along p, 4x tiled
        nc.sync.dma_start(slbc[:], ext["slbc"][:, :])

        # per-k-chunk DMAs: first gate matmuls start before the full set lands
        w0 = cpool.tile([128, 2 * KH, G], F8)
        for k in range(2 * KH):
            nc.sync.dma_start(w0[:, k, :], ext["w0"][k, :, :])
        w1 = cpool.tile([128, 2 * KH, G], F8)
        for k in range(2 * KH):
            nc.sync.dma_start(w1[:, k, :], ext["w1"][k, :, :])
        wsm = cpool.tile([128, KH, VSH], F8)
        nc.sync.dma_start(wsm[:], ext["wsm"][:, :, :].rearrange("k p n -> p k n"))
        # pre-transposed, pre-scaled embedded inputs and label rows
        tch = min(8, T)
        xt_all = cpool.tile([128, T, KH, B], F8)
        for tt_ in range(0, T, tch):
            nc.sync.dma_start(xt_all[:, tt_:tt_ + tch, :, :],
                              ext["xt8"][:, tt_:tt_ + tch, :, :])
        wl_all = cpool.tile([128, T, H], F8)
        for tt_ in range(0, T, tch):
            nc.sync.dma_start(wl_all[:, tt_:tt_ + tch, :],
                              ext["wl8"][:, tt_:tt_ + tch, :])

        identB = cpool.tile([128, 128], MM_DT)
        make_identity(nc, identB[:])

        Sacc = cpool.tile([B, T], f32)
        LDacc = cpool.tile([B, T], f32)

        # ---- initial states ------------------------------------------------
        c_st = {}
        hT_st = {}
        for li in (0, 1):
            c_st[li] = state.tile([B, H], f32, name=f"c{li}", tag=f"c{li}")
            nc.vector.memset(c_st[li][:], 0.0)
            hT_st[li] = state.tile([128, H], F8, name=f"hT{li}", tag=f"hT{li}")
            nc.vector.memset(hT_st[li][:], 0.0)

        def alloc_gates(li):
            # layer-0 pairs double-buffer across steps (x-part of t+1 runs
            # while t is consumed); layer-1 needs only one live pair.
            return [gpsum.tile([B, G // 2], f32, name=f"g{li}", tag=f"g{li}",
                               bufs=(2 if li == 0 else 1))
                    for _ in (0, 1)]

        def gates_part(halves, lhs_fn, w_tile, part, start, stop):
            # half-outer: each [B,1024] PSUM half finishes its accumulation
            # early so the cell's ACT read starts sooner.  DoubleRow: each
            # (half, n) slice accumulates K=512 as two K=256 chunks.
            k0 = 0 if part == "x" else KH
            for half in (0, 1):
                gh = halves[half]
                for d in (0, 1):
                    lhs = lhs_fn(d)
                    for n in (0, 1):
                        sl = slice(512 * n, 512 * (n + 1))
                        wsl = slice(1024 * half + 512 * n,
                                    1024 * half + 512 * (n + 1))
                        nc.tensor.matmul(gh[:, sl], lhs,
                                         w_tile[:, k0 + 2 * d:k0 + 2 * d + 2, wsl],
                                         start=(start and d == 0),
                                         stop=(stop and d == 1),
                                         perf_mode=DR)

        def cell(t, li, ghalves):
            """LSTM cell elementwise chain. gates ordered [f, i | o, cg].

            All gate columns need the same Tanh(g/256) (sigmoid 0.5 factors
            are folded into the f/i/o weight columns), so each PSUM half is
            one ACT op.  Returns o8 = 8*m*h_new (bf16, B-layout)."""
            gA, gB = ghalves
            mht = Mh[:, t:t + 1]
            m1ht = M1h[:, t:t + 1]
            m4t = M4[:, t:t + 1]

            thA = wp.tile([B, G // 2], MM_DT, name="thA", tag="thA")
            nc.scalar.activation(thA[:], gA[:], AF.Tanh, scale=GS)
            thB = wp.tile([B, G // 2], MM_DT, name="thB", tag="thB")
            nc.scalar.activation(thB[:], gB[:], AF.Tanh, scale=GS)

            fp = wp.tile([B, H], MM_DT, name="fp", tag="fp")
            nc.vector.tensor_scalar(out=fp[:], in0=thA[:, 0:512], scalar1=mht,
                                    scalar2=m1ht, op0=OP.mult, op1=OP.add)
            ip = wp.tile([B, H], MM_DT, name="ip", tag="ip")
            nc.vector.tensor_scalar(out=ip[:], in0=thA[:, 512:1024], scalar1=mht,
                                    scalar2=mht, op0=OP.mult, op1=OP.add)
            osm = wp.tile([B, H], MM_DT, name="osm", tag="osm")
            nc.vector.tensor_scalar(out=osm[:], in0=thB[:, 0:512], scalar1=m4t,
                                    scalar2=m4t, op0=OP.mult, op1=OP.add)

            r = wp.tile([B, H], f32, name="r", tag="r")
            nc.vector.tensor_tensor(out=r[:], in0=fp[:], in1=c_st[li][:],
                                    op=OP.mult)
            q = wp.tile([B, H], MM_DT, name="q", tag="q")
            nc.vector.tensor_tensor(out=q[:], in0=ip[:], in1=thB[:, 512:1024],
                                    op=OP.mult)
            c_new = state.tile([B, H], f32, name=f"c{li}", tag=f"c{li}")
            nc.vector.tensor_tensor(out=c_new[:], in0=r[:], in1=q[:], op=OP.add)
            c_st[li] = c_new
            tc_ = wp.tile([B, H], MM_DT, name="tc_", tag="tc_")
            nc.scalar.activation(tc_[:], c_new[:], AF.Tanh)
            o8 = wp.tile([B, H], MM_DT, name=f"o{li}", tag=f"o{li}")
            nc.vector.tensor_tensor(out=o8[:], in0=osm[:], in1=tc_[:],
                                    op=OP.mult)
            return o8

        def transpose_o(o8, li):
            ps = tpsum.tile([128, H], MM_DT, name="tp", tag="tp")
            for kc in range(KH):
                sl = slice(128 * kc, 128 * (kc + 1))
                nc.tensor.transpose(ps[:, sl], o8[:, sl], identB[:])
            oT = wp.tile([128, H], F8, name=f"oT{li}", tag=f"oT{li}")
            nc.scalar.copy(oT[:, 0:256], ps[:, 0:256])
            nc.vector.tensor_copy(oT[:, 256:512], ps[:, 256:512])
            return oT

        def update_hT(li, oT, m1bc):
            # hT_new = (1-m)*hT_old + oT   (exact freeze for finished rows)
            tmp = wp.tile([128, H], MM_DT, name="htmp", tag="htmp")
            nc.vector.tensor_tensor(out=tmp[:], in0=hT_st[li][:], in1=m1bc[:],
                                    op=OP.mult)
            hT = state.tile([128, H], F8, name=f"hT{li}", tag=f"hT{li}")
            nc.vector.tensor_tensor(out=hT[:], in0=tmp[:], in1=oT[:], op=OP.add)
            hT_st[li] = hT

        def project(t, o1T8, o1_8):
            sps = []
            for (n0, n1) in PCHUNKS:
                w = n1 - n0
                pp = ppsum.tile([128, 512], f32, name="pp", tag="pp")
                for d in (0, 1):
                    nc.tensor.matmul(pp[:, 0:w], _dr2(o1T8[:], d),
                                     wsm[:, 2 * d:2 * d + 2, n0:n1],
                                     start=(d == 0), stop=(d == 1),
                                     perf_mode=DR)
                sp_i = wp.tile([B, 1], f32, name="sp_i", tag="sp_i")
                exp_scr = wp.tile([B, 512], MM_DT, name="exp_scr", tag="exp_scr")
                nc.scalar.activation(exp_scr[:, 0:w], pp[:, 0:w], AF.Exp,
                                     scale=GS, accum_out=sp_i[:])
                sps.append(sp_i)
            s01 = wp.tile([B, 1], f32, name="s01", tag="s01")
            nc.vector.tensor_tensor(out=s01[:], in0=sps[0][:], in1=sps[1][:],
                                    op=OP.add)
            nc.vector.tensor_tensor(out=Sacc[:, t:t + 1], in0=s01[:],
                                    in1=sps[2][:], op=OP.add)
            ld_scr = wp.tile([B, H], MM_DT, name="ld_scr", tag="ld_scr")
            nc.vector.scalar_tensor_tensor(out=ld_scr[:], in0=o1_8[:],
                                           scalar=GS, in1=wl_all[:, t, :],
                                           op0=OP.mult, op1=OP.mult,
                                           accum_out=LDacc[:, t:t + 1])

        # ---- software-pipelined main loop ---------------------------------
        # Emission order = per-engine issue order.  Keep ready MM work (next
        # step's x-part, previous step's projection) queued on the PE while
        # the serial cell chains run on ACT/DVE, so the PE never idles past
        # the HAM re-throttle window.
        def xt_lhs(t):
            return lambda d: xt_all[:, t, 2 * d:2 * d + 2, :]

        g0 = alloc_gates(0)
        gates_part(g0, xt_lhs(0), w0, "x", start=True, stop=True)  # t=0: no rec
        o1_prev = None
        o1T_prev = None
        for t in range(T):
            # (1-m[t]) broadcast along partitions, tiled 4x: for hT updates
            if t + 1 < T:
                m1bc = wp.tile([128, H], MM_DT, name="m1bc", tag="m1bc")
                nc.vector.tensor_scalar(out=m1bc[:], in0=slbc[:],
                                        scalar1=float(t), scalar2=None,
                                        op0=OP.is_le)
            g1 = None
            if t > 0:
                g1 = alloc_gates(1)
                gates_part(g1, lambda d: _dr2(hT_st[1][:], d), w1, "h",
                           start=True, stop=False)
            o0 = cell(t, 0, g0)
            if t + 1 < T:
                g0 = alloc_gates(0)
                gates_part(g0, xt_lhs(t + 1), w0, "x", start=True, stop=False)
            o0T = transpose_o(o0, 0)
            if g1 is None:
                g1 = alloc_gates(1)
                gates_part(g1, lambda d: _dr2(o0T[:], d), w1, "x",
                           start=True, stop=True)
            else:
                gates_part(g1, lambda d: _dr2(o0T[:], d), w1, "x",
                           start=False, stop=True)
            if t + 1 < T:
                update_hT(0, o0T, m1bc)
            if o1T_prev is not None:
                project(t - 1, o1T_prev, o1_prev)
            o1 = cell(t, 1, g1)
            if t + 1 < T:
                gates_part(g0, lambda d: _dr2(hT_st[0][:], d), w0, "h",
                           start=False, stop=True)
            o1T = transpose_o(o1, 1)
            if t + 1 < T:
                update_hT(1, o1T, m1bc)
            o1_prev, o1T_prev = o1, o1T
        project(T - 1, o1T_prev, o1_prev)

        # gate LD to core 0 (ldsc = 1 there, 0 elsewhere)
        nc.vector.tensor_scalar(out=LDacc[:], in0=LDacc[:],
                                scalar1=ldsc[:, 0:1], scalar2=None,
                                op0=OP.mult)
        nc.sync.dma_start(ext["S"][:, :], Sacc[:])
        nc.sync.dma_start(ext["LD"][:, :], LDacc[:])


def _build():
    if "nc" in _CACHE:
        return _CACHE["nc"]
    nc = bacc.Bacc("TRN2", target_bir_lowering=False, debug=False,
                   num_devices=NCORES)
    ext = {
        "ldsc": nc.declare_dram_parameter("ldsc", [B, 1], dt.float32,
                                          isOutput=False),
        "Mh": nc.declare_dram_parameter("Mh", [B, T], dt.float32,
                                        isOutput=False),
        "M1h": nc.declare_dram_parameter("M1h", [B, T], dt.float32,
                                         isOutput=False),
        "M4": nc.declare_dram_parameter("M4", [B, T], dt.float32,
                                        isOutput=False),
        "slbc": nc.declare_dram_parameter("slbc", [128, H], MM_DT,
                                          isOutput=False),
        "w0": nc.declare_dram_parameter("w0", [2 * KH, 128, G], F8,
                                        isOutput=False),
        "w1": nc.declare_dram_parameter("w1", [2 * KH, 128, G], F8,
                                        isOutput=False),
        "wsm": nc.declare_dram_parameter("wsm", [KH, 128, VSH], F8,
                                         isOutput=False),
        "xt8": nc.declare_dram_parameter("xt8", [128, T, KH, B], F8,
                                         isOutput=False),
        "wl8": nc.declare_dram_parameter("wl8", [128, T, H], F8,
                                         isOutput=False),
        "S": nc.declare_dram_parameter("S", [B, T], dt.float32, isOutput=True),
        "LD": nc.declare_dram_parameter("LD", [B, T], dt.float32, isOutput=True),
    }
    with tile.TileContext(nc) as tc:
        _emit(nc, tc, ext)
    nc.compile()
    _CACHE["nc"] = nc
    return nc


def _pack_w(Wx, Wh):
    w = np.concatenate([np.asarray(Wx, np.float32), np.asarray(Wh, np.float32)],
                       axis=0)  # [2H, 4H] rows: x-part then h-part
    # reorder gate blocks [i, cg, f, o] -> [f, i, o, cg]
    w = np.concatenate([w[:, 1024:1536], w[:, 0:512], w[:, 1536:2048],
                        w[:, 512:1024]], axis=1)
    # sigmoid trick: tanh(0.5*g) for f/i/o -> fold 0.5 into those columns
    w[:, 0:1536] *= 0.5
    w *= SW
    return np.ascontiguousarray(w.reshape(2 * KH, 128, G)).astype(NP_F8)


def kernel(features, labels, seq_lengths, seq_mask, embedding,
           W0x, W0h, b0, W1x, W1h, b1, softmax_w, softmax_b,
           _trace_dir=None):
    for name, b in (("b0", b0), ("b1", b1), ("softmax_b", softmax_b)):
        if np.any(np.asarray(b, np.float32) != 0.0):
            raise NotImplementedError(f"{name} != 0 not supported")

    feats = np.asarray(features, np.int64)[:, :T]
    labs = np.asarray(labels, np.int64)[:, :T]
    slen = np.asarray(seq_lengths, np.int32).astype(np.float32)  # [B]
    mask = np.asarray(seq_mask, np.float32)[:, :T]
    emb = np.asarray(embedding, np.float32)
    w0 = _pack_w(W0x, W0h)
    w1 = _pack_w(W1x, W1h)
    wsm_f = np.asarray(softmax_w, np.float32) * SW
    wsm_r = wsm_f.reshape(KH, 128, V)

    # host-side data prep (layout only, no model math):
    # xt8[p, t, kc, b] = SH * emb[features[b, t], kc*128 + p]
    x = emb[feats] * SH                              # [B, T, H]
    xt8 = np.ascontiguousarray(
        x.reshape(B, T, KH, 128).transpose(3, 1, 2, 0)).astype(NP_F8)
    # wl8[b, t, :] = SW * softmax_w[:, labels[b, t]]
    wl8 = np.ascontiguousarray(
        wsm_f.T[labs].transpose(0, 1, 2)).astype(NP_F8)  # [B, T, H]
    # masks
    m = (np.arange(T)[None, :] < slen[:, None]).astype(np.float32)  # [B, T]
    Mh = 0.5 * m
    M1h = 1.0 - 0.5 * m
    M4 = 4.0 * m
    # slbc[p, kc*128+b] = slen[b]
    slbc = np.ascontiguousarray(
        np.tile(slen[None, None, :], (128, KH, 1)).reshape(128, H)
    ).astype(NP_BF16)

    nc = _build()
    in_maps = []
    for c in range(NCORES):
        in_maps.append({
            "ldsc": np.full((B, 1), 1.0 if c == 0 else 0.0, np.float32),
            "Mh": Mh, "M1h": M1h, "M4": M4, "slbc": slbc,
            "w0": w0,
            "w1": w1,
            "wsm": np.ascontiguousarray(
                wsm_r[:, :, c * VSH:(c + 1) * VSH]).astype(NP_F8),
            "xt8": xt8,
            "wl8": wl8,
        })

    kwargs = {}
    if _trace_dir is not None:
        kwargs = dict(trace=True, tmpdir=_trace_dir)
    res = run_bass_kernel_spmd(nc, in_maps, list(range(NCORES)), **kwargs)
    _CACHE["last_results"] = res

    S = np.zeros((B, T), np.float64)
    LD = np.zeros((B, T), np.float64)
    for c in range(NCORES):
        S += np.asarray(res.results[c]["S"], np.float64)
        LD += np.asarray(res.results[c]["LD"], np.float64)

    xent = np.log(S) - LD
    loss_t = (xent * mask).sum(axis=0) / (mask.sum(axis=0) + 1e-12)
    cost = loss_t.mean()
    return np.asarray(cost, np.float32)


# revision 10
# speedup vs baseline: 1.7935x; 1.0411x over previous
"""Trainium2 Bass kernel: 2-layer LSTM language-model loss (fp8 DoubleRow).

Reference: x = embedding[features]; 2-layer LSTM over T=64 steps with
sequence-length state freezing; logits = out @ softmax_w + softmax_b;
masked mean cross-entropy -> scalar.

Strategy (8 NeuronCores, SPMD, zero cross-core collectives):
  * Every core runs the identical full-batch (B=128) recurrence; the
    large projection is sharded over the vocab dim (core c owns columns
    [c*1250, (c+1)*1250)), producing per (b,t) S_c = sum_v exp(logit_v).
    The label logit is a row-wise dot with a host-pregathered row of
    softmax_w^T, gated to core 0.  Host: xent = log(sum_c S_c) - LD.
  * All matmuls run fp8e4m3 + MatmulPerfMode.DoubleRow (K virtualized
    to 256 -> ~1.8x the bf16 streaming rate).  Scales: stationaries
    (x^T, o^T, h^T) carry SH=8x, weights SW=32x (with the sigmoid-trick
    0.5 folded into f/i/o weight columns), undone by ACT scale 1/256.
  * Host pre-gathers/transposes the embedded inputs (xt8) and the label
    rows of softmax_w^T (wl8) -- both are pure data layout, no FLOPs --
    so the loop has no indirect DMA and no PE transposes for x.
  * The h state lives ONLY in transposed (stationary) layout:
    hT_new = (1-m)*hT_old + oT, with (1-m) broadcast along partitions
    from a host-shipped slen row (slbc).  No h transposes.
  * Emission order keeps the PE fed with ready work (next step's x-part
    gates, the previous step's projection) while the serial cell chain
    runs on ACT/DVE, so the HAM clock gate stays at 2.4 GHz.

Assumes b0 = b1 = softmax_b = 0 (verified at runtime).
"""

import numpy as np
import ml_dtypes


def _ensure_path():
    try:
        import concourse  # noqa: F401
    except ImportError:
        import sys

        for p in ("/opt/trn_rl_repo", "/root/.axon_site/_ro/trn_rl_repo"):
            if p not in sys.path:
                sys.path.append(p)


_ensure_path()

from contextlib import ExitStack  # noqa: E402

import concourse.bass as bass  # noqa: E402
import concourse.bacc as bacc  # noqa: E402
import concourse.tile as tile  # noqa: E402
from concourse import mybir  # noqa: E402
from concourse.alu_op_type import AluOpType as OP  # noqa: E402
from concourse.bass_utils import run_bass_kernel_spmd  # noqa: E402
from concourse.masks import make_identity  # noqa: E402

dt = mybir.dt
AF = mybir.ActivationFunctionType
DR = mybir.MatmulPerfMode.DoubleRow

import os as _os

B = 128
T = int(_os.environ.get("KERNEL_T_OVERRIDE", "64"))
H = 512
V = 10000
NCORES = 8
VSH = V // NCORES  # 1250
G = 4 * H  # 2048
KH = H // 128  # 4 k-chunks per 512-wide contraction
MM_DT = dt.bfloat16
F8 = dt.float8e4
NP_F8 = ml_dtypes.float8_e4m3
NP_BF16 = ml_dtypes.bfloat16
SW = 32.0   # weight scale into fp8
SH = 8.0    # stationary-activation scale into fp8
GS = 1.0 / (SW * SH)  # activation un-scale
# projection free-dim chunks (PSUM bank = 512 fp32)
PCHUNKS = [(0, 512), (512, 1024), (1024, VSH)]

_CACHE: dict = {}


def _dr2(ap512, d):
    """[128, 512] T-layout slice for DoubleRow chunk d: [128, 2, 128]."""
    return ap512[:, 256 * d:256 * (d + 1)].rearrange("p (two m) -> p two m",
                                                     two=2)


def _emit(nc, tc, ext):
    f32 = dt.float32
    with ExitStack() as ctx:
        cpool = ctx.enter_context(tc.tile_pool(name="const", bufs=1))
        state = ctx.enter_context(tc.tile_pool(name="state", bufs=2))
        wp = ctx.enter_context(tc.tile_pool(name="work", bufs=3))
        gpsum = ctx.enter_context(tc.tile_pool(name="gpsum", bufs=2, space="PSUM"))
        tpsum = ctx.enter_context(tc.tile_pool(name="tpsum", bufs=1, space="PSUM"))
        ppsum = ctx.enter_context(tc.tile_pool(name="ppsum", bufs=1, space="PSUM"))

        # ---- constants / inputs -------------------------------------------
        ldsc = cpool.tile([B, 1], f32)
        nc.sync.dma_start(ldsc[:], ext["ldsc"][:, :])
        Mh = cpool.tile([B, T], f32)   # 0.5*m
        nc.sync.dma_start(Mh[:], ext["Mh"][:, :])
        M1h = cpool.tile([B, T], f32)  # 1-0.5*m
        nc.sync.dma_start(M1h[:], ext["M1h"][:, :])
        M4 = cpool.tile([B, T], f32)   # 4*m
        nc.sync.dma_start(M4[:], ext["M4"][:, :])
        slbc = cpool.tile([128, H], MM_DT)  # slen[b] bcast along p, 4x tiled
        nc.sync.dma_start(slbc[:], ext["slbc"][:, :])

        # per-k-chunk DMAs: first gate matmuls start before the full set lands
        w0 = cpool.tile([128, 2 * KH, G], F8)
        for k in range(2 * KH):
            nc.sync.dma_start(w0[:, k, :], ext["w0"][k, :, :])
        w1 = cpool.tile([128, 2 * KH, G], F8)
        for k in range(2 * KH):
            nc.sync.dma_start(w1[:, k, :], ext["w1"][k, :, :])
        wsm = cpool.tile([128, KH, VSH], F8)
        nc.sync.dma_start(wsm[:], ext["wsm"][:, :, :].rearrange("k p n -> p k n"))
        # pre-transposed, pre-scaled embedded inputs and label rows
        tch = min(8, T)
        xt_all = cpool.tile([128, T, KH, B], F8)
        for tt_ in range(0, T, tch):
            nc.sync.dma_start(xt_all[:, tt_:tt_ + tch, :, :],
                              ext["xt8"][:, tt_:tt_ + tch, :, :])
        wl_all = cpool.tile([128, T, H], F8)
        for tt_ in range(0, T, tch):
            nc.sync.dma_start(wl_all[:, tt_:tt_ + tch, :],
                              ext["wl8"][:, tt_:tt_ + tch, :])

        identB = cpool.tile([128, 128], MM_DT)
        make_identity(nc, identB[:])

        Sacc = cpool.tile([B, T], f32)
        LDacc = cpool.tile([B, T], f32)

        # ---- initial states ------------------------------------------------
        c_st = {}
        hT_st = {}
        for li in (0, 1):
            c_st[li] = state.tile([B, H], f32, name=f"c{li}", tag=f"c{li}")
            nc.vector.memset(c_st[li][:], 0.0)
            hT_st[li] = state.tile([128, H], F8, name=f"hT{li}", tag=f"hT{li}")
            nc.vector.memset(hT_st[li][:], 0.0)

        def alloc_gates(li):
            # layer-0 pairs double-buffer across steps (x-part of t+1 runs
            # while t is consumed); layer-1 needs only one live pair.
            return [gpsum.tile([B, G // 2], f32, name=f"g{li}", tag=f"g{li}",
                               bufs=(2 if li == 0 else 1))
                    for _ in (0, 1)]

        def gates_part(halves, lhs_fn, w_tile, part, start, stop):
            # half-outer: each [B,1024] PSUM half finishes its accumulation
            # early so the cell's ACT read starts sooner.  DoubleRow: each
            # (half, n) slice accumulates K=512 as two K=256 chunks.
            k0 = 0 if part == "x" else KH
            for half in (0, 1):
                gh = halves[half]
                for d in (0, 1):
                    lhs = lhs_fn(d)
                    for n in (0, 1):
                        sl = slice(512 * n, 512 * (n + 1))
                        wsl = slice(1024 * half + 512 * n,
                                    1024 * half + 512 * (n + 1))
                        nc.tensor.matmul(gh[:, sl], lhs,
                                         w_tile[:, k0 + 2 * d:k0 + 2 * d + 2, wsl],
                                         start=(start and d == 0),
                                         stop=(stop and d == 1),
                                         perf_mode=DR)

        def cell(t, li, ghalves):
            """LSTM cell elementwise chain. gates ordered [f, i | o, cg].

            All gate columns need the same Tanh(g/256) (sigmoid 0.5 factors
            are folded into the f/i/o weight columns), so each PSUM half is
            one ACT op.  Returns o8 = 8*m*h_new (bf16, B-layout)."""
            gA, gB = ghalves
            mht = Mh[:, t:t + 1]
            m1ht = M1h[:, t:t + 1]
            m4t = M4[:, t:t + 1]

            # split the gate halves into 512-col ACT slices so the DVE chain
            # starts after the first slice, shortening the serial chain
            thA = wp.tile([B, G // 2], MM_DT, name="thA", tag="thA")
            nc.scalar.activation(thA[:, 0:512], gA[:, 0:512], AF.Tanh, scale=GS)
            fp = wp.tile([B, H], MM_DT, name="fp", tag="fp")
            nc.vector.tensor_scalar(out=fp[:], in0=thA[:, 0:512], scalar1=mht,
                                    scalar2=m1ht, op0=OP.mult, op1=OP.add)
            nc.scalar.activation(thA[:, 512:1024], gA[:, 512:1024], AF.Tanh,
                                 scale=GS)
            ip = wp.tile([B, H], MM_DT, name="ip", tag="ip")
            nc.vector.tensor_scalar(out=ip[:], in0=thA[:, 512:1024], scalar1=mht,
                                    scalar2=mht, op0=OP.mult, op1=OP.add)
            thB = wp.tile([B, G // 2], MM_DT, name="thB", tag="thB")
            nc.scalar.activation(thB[:, 512:1024], gB[:, 512:1024], AF.Tanh,
                                 scale=GS)
            r = wp.tile([B, H], f32, name="r", tag="r")
            nc.vector.tensor_tensor(out=r[:], in0=fp[:], in1=c_st[li][:],
                                    op=OP.mult)
            q = wp.tile([B, H], MM_DT, name="q", tag="q")
            nc.vector.tensor_tensor(out=q[:], in0=ip[:], in1=thB[:, 512:1024],
                                    op=OP.mult)
            nc.scalar.activation(thB[:, 0:512], gB[:, 0:512], AF.Tanh, scale=GS)
            osm = wp.tile([B, H], MM_DT, name="osm", tag="osm")
            nc.vector.tensor_scalar(out=osm[:], in0=thB[:, 0:512], scalar1=m4t,
                                    scalar2=m4t, op0=OP.mult, op1=OP.add)
            c_new = state.tile([B, H], f32, name=f"c{li}", tag=f"c{li}")
            o8 = wp.tile([B, H], MM_DT, name=f"o{li}", tag=f"o{li}")
            tc_ = wp.tile([B, H], MM_DT, name="tc_", tag="tc_")
            # halves: tanh(c) and o emerge 256 cols at a time -> transposes
            # and the layer-1 x gates start half a chain earlier
            for hf in (0, 1):
                sl = slice(256 * hf, 256 * (hf + 1))
                nc.vector.tensor_tensor(out=c_new[:, sl], in0=r[:, sl],
                                        in1=q[:, sl], op=OP.add)
                nc.scalar.activation(tc_[:, sl], c_new[:, sl], AF.Tanh)
                nc.vector.tensor_tensor(out=o8[:, sl], in0=osm[:, sl],
                                        in1=tc_[:, sl], op=OP.mult)
            c_st[li] = c_new
            return o8

        def transpose_o(o8, li):
            ps = tpsum.tile([128, H], MM_DT, name="tp", tag="tp")
            for kc in range(KH):
                sl = slice(128 * kc, 128 * (kc + 1))
                nc.tensor.transpose(ps[:, sl], o8[:, sl], identB[:])
            oT = wp.tile([128, H], F8, name=f"oT{li}", tag=f"oT{li}")
            nc.scalar.copy(oT[:, 0:256], ps[:, 0:256])
            nc.vector.tensor_copy(oT[:, 256:512], ps[:, 256:512])
            return oT

        def update_hT(li, oT, m1bc):
            # hT_new = (1-m)*hT_old + oT   (exact freeze for finished rows)
            tmp = wp.tile([128, H], MM_DT, name="htmp", tag="htmp")
            nc.vector.tensor_tensor(out=tmp[:], in0=hT_st[li][:], in1=m1bc[:],
                                    op=OP.mult)
            hT = state.tile([128, H], F8, name=f"hT{li}", tag=f"hT{li}")
            nc.vector.tensor_tensor(out=hT[:], in0=tmp[:], in1=oT[:], op=OP.add)
            hT_st[li] = hT

        def project(t, o1T8, o1_8):
            sps = []
            for (n0, n1) in PCHUNKS:
                w = n1 - n0
                pp = ppsum.tile([128, 512], f32, name="pp", tag="pp")
                for d in (0, 1):
                    nc.tensor.matmul(pp[:, 0:w], _dr2(o1T8[:], d),
                                     wsm[:, 2 * d:2 * d + 2, n0:n1],
                                     start=(d == 0), stop=(d == 1),
                                     perf_mode=DR)
                sp_i = wp.tile([B, 1], f32, name="sp_i", tag="sp_i")
                exp_scr = wp.tile([B, 512], MM_DT, name="exp_scr", tag="exp_scr")
                nc.scalar.activation(exp_scr[:, 0:w], pp[:, 0:w], AF.Exp,
                                     scale=GS, accum_out=sp_i[:])
                sps.append(sp_i)
            s01 = wp.tile([B, 1], f32, name="s01", tag="s01")
            nc.vector.tensor_tensor(out=s01[:], in0=sps[0][:], in1=sps[1][:],
                                    op=OP.add)
            nc.vector.tensor_tensor(out=Sacc[:, t:t + 1], in0=s01[:],
                                    in1=sps[2][:], op=OP.add)
            ld_scr = wp.tile([B, H], MM_DT, name="ld_scr", tag="ld_scr")
            nc.vector.scalar_tensor_tensor(out=ld_scr[:], in0=o1_8[:],
                                           scalar=GS, in1=wl_all[:, t, :],
                                           op0=OP.mult, op1=OP.mult,
                                           accum_out=LDacc[:, t:t + 1])

        # ---- software-pipelined main loop ---------------------------------
        # Emission order = per-engine issue order.  Keep ready MM work (next
        # step's x-part, previous step's projection) queued on the PE while
        # the serial cell chains run on ACT/DVE, so the PE never idles past
        # the HAM re-throttle window.
        def xt_lhs(t):
            return lambda d: xt_all[:, t, 2 * d:2 * d + 2, :]

        g0 = alloc_gates(0)
        gates_part(g0, xt_lhs(0), w0, "x", start=True, stop=True)  # t=0: no rec
        o1_prev = None
        o1T_prev = None
        for t in range(T):
            # (1-m[t]) broadcast along partitions, tiled 4x: for hT updates
            if t + 1 < T:
                m1bc = wp.tile([128, H], MM_DT, name="m1bc", tag="m1bc")
                nc.vector.tensor_scalar(out=m1bc[:], in0=slbc[:],
                                        scalar1=float(t), scalar2=None,
                                        op0=OP.is_le)
            g1 = None
            if t > 0:
                g1 = alloc_gates(1)
                gates_part(g1, lambda d: _dr2(hT_st[1][:], d), w1, "h",
                           start=True, stop=False)
            o0 = cell(t, 0, g0)
            if t + 1 < T:
                g0 = alloc_gates(0)
                gates_part(g0, xt_lhs(t + 1), w0, "x", start=True, stop=False)
            o0T = transpose_o(o0, 0)
            if g1 is None:
                g1 = alloc_gates(1)
                gates_part(g1, lambda d: _dr2(o0T[:], d), w1, "x",
                           start=True, stop=True)
            else:
                gates_part(g1, lambda d: _dr2(o0T[:], d), w1, "x",
                           start=False, stop=True)
            if t + 1 < T:
                update_hT(0, o0T, m1bc)
            if o1T_prev is not None:
                project(t - 1, o1T_prev, o1_prev)
            o1 = cell(t, 1, g1)
            if t + 1 < T:
                gates_part(g0, lambda d: _dr2(hT_st[0][:], d), w0, "h",
                           start=False, stop=True)
            o1T = transpose_o(o1, 1)
            if t + 1 < T:
                update_hT(1, o1T, m1bc)
            o1_prev, o1T_prev = o1, o1T
        project(T - 1, o1T_prev, o1_prev)

        # gate LD to core 0 (ldsc = 1 there, 0 elsewhere)
        nc.vector.tensor_scalar(out=LDacc[:], in0=LDacc[:],
                                scalar1=ldsc[:, 0:1], scalar2=None,
                                op0=OP.mult)
        nc.sync.dma_start(ext["S"][:, :], Sacc[:])
        nc.sync.dma_start(ext["LD"][:, :], LDacc[:])


def _build():
    if "nc" in _CACHE:
        return _CACHE["nc"]
    nc = bacc.Bacc("TRN2", target_bir_lowering=False, debug=False,
                   num_devices=NCORES)
    ext = {
        "ldsc": nc.declare_dram_parameter("ldsc", [B, 1], dt.float32,
                                          isOutput=False),
        "Mh": nc.declare_dram_parameter("Mh", [B, T], dt.float32,
                                        isOutput=False),
        "M1h": nc.declare_dram_parameter("M1h", [B, T], dt.float32,
                                         isOutput=False),
        "M4": nc.declare_dram_parameter("M4", [B, T], dt.float32,
                                        isOutput=False),
        "slbc": nc.declare_dram_parameter("slbc", [128, H], MM_DT,
                                          isOutput=False),
        "w0": nc.declare_dram_parameter("w0", [2 * KH, 128, G], F8,
                                        isOutput=False),
        "w1": nc.declare_dram_parameter("w1", [2 * KH, 128, G], F8,
                                        isOutput=False),
        "wsm": nc.declare_dram_parameter("wsm", [KH, 128, VSH], F8,
                                         isOutput=False),
        "xt8": nc.declare_dram_parameter("xt8", [128, T, KH, B], F8,
                                         isOutput=False),
        "wl8": nc.declare_dram_parameter("wl8", [128, T, H], F8,
                                         isOutput=False),
        "S": nc.declare_dram_parameter("S", [B, T], dt.float32, isOutput=True),
        "LD": nc.declare_dram_parameter("LD", [B, T], dt.float32, isOutput=True),
    }
    with tile.TileContext(nc) as tc:
        _emit(nc, tc, ext)
    nc.compile()
    _CACHE["nc"] = nc
    return nc


def _pack_w(Wx, Wh):
    w = np.concatenate([np.asarray(Wx, np.float32), np.asarray(Wh, np.float32)],
                       axis=0)  # [2H, 4H] rows: x-part then h-part
    # reorder gate blocks [i, cg, f, o] -> [f, i, o, cg]
    w = np.concatenate([w[:, 1024:1536], w[:, 0:512], w[:, 1536:2048],
                        w[:, 512:1024]], axis=1)
    # sigmoid trick: tanh(0.5*g) for f/i/o -> fold 0.5 into those columns
    w[:, 0:1536] *= 0.5
    w *= SW
    return np.ascontiguousarray(w.reshape(2 * KH, 128, G)).astype(NP_F8)


def kernel(features, labels, seq_lengths, seq_mask, embedding,
           W0x, W0h, b0, W1x, W1h, b1, softmax_w, softmax_b,
           _trace_dir=None):
    for name, b in (("b0", b0), ("b1", b1), ("softmax_b", softmax_b)):
        if np.any(np.asarray(b, np.float32) != 0.0):
            raise NotImplementedError(f"{name} != 0 not supported")

    feats = np.asarray(features, np.int64)[:, :T]
    labs = np.asarray(labels, np.int64)[:, :T]
    slen = np.asarray(seq_lengths, np.int32).astype(np.float32)  # [B]
    mask = np.asarray(seq_mask, np.float32)[:, :T]
    emb = np.asarray(embedding, np.float32)
    w0 = _pack_w(W0x, W0h)
    w1 = _pack_w(W1x, W1h)
    wsm_f = np.asarray(softmax_w, np.float32) * SW
    wsm_r = wsm_f.reshape(KH, 128, V)

    # host-side data prep (layout only, no model math):
    # xt8[p, t, kc, b] = SH * emb[features[b, t], kc*128 + p]
    x = emb[feats] * SH                              # [B, T, H]
    xt8 = np.ascontiguousarray(
        x.reshape(B, T, KH, 128).transpose(3, 1, 2, 0)).astype(NP_F8)
    # wl8[b, t, :] = SW * softmax_w[:, labels[b, t]]
    wl8 = np.ascontiguousarray(
        wsm_f.T[labs].transpose(0, 1, 2)).astype(NP_F8)  # [B, T, H]
    # masks
    m = (np.arange(T)[None, :] < slen[:, None]).astype(np.float32)  # [B, T]
    Mh = 0.5 * m
    M1h = 1.0 - 0.5 * m
    M4 = 4.0 * m
    # slbc[p, kc*128+b] = slen[b]
    slbc = np.ascontiguousarray(
        np.tile(slen[None, None, :], (128, KH, 1)).reshape(128, H)
    ).astype(NP_BF16)

    nc = _build()
    in_maps = []
    for c in range(NCORES):
        in_maps.append({
            "ldsc": np.full((B, 1), 1.0 if c == 0 else 0.0, np.float32),
            "Mh": Mh, "M1h": M1h, "M4": M4, "slbc": slbc,
            "w0": w0,
            "w1": w1,
            "wsm": np.ascontiguousarray(
                wsm_r[:, :, c * VSH:(c + 1) * VSH]).astype(NP_F8),
            "xt8": xt8,
            "wl8": wl8,
        })

    kwargs = {}
    if _trace_dir is not None:
        kwargs = dict(trace=True, tmpdir=_trace_dir)
    res = run_bass_kernel_spmd(nc, in_maps, list(range(NCORES)), **kwargs)
    _CACHE["last_results"] = res

    S = np.zeros((B, T), np.float64)
    LD = np.zeros((B, T), np.float64)
    for c in range(NCORES):
        S += np.asarray(res.results[c]["S"], np.float64)
        LD += np.asarray(res.results[c]["LD"], np.float64)

    xent = np.log(S) - LD
    loss_t = (xent * mask).sum(axis=0) / (mask.sum(axis=0) + 1e-12)
    cost = loss_t.mean()
    return np.asarray(cost, np.float32)


# revision 12
# speedup vs baseline: 1.7963x; 1.0015x over previous
"""Trainium2 Bass kernel: 2-layer LSTM language-model loss (fp8 DoubleRow).

Reference: x = embedding[features]; 2-layer LSTM over T=64 steps with
sequence-length state freezing; logits = out @ softmax_w + softmax_b;
masked mean cross-entropy -> scalar.

Strategy (8 NeuronCores, SPMD, zero cross-core collectives):
  * Every core runs the identical full-batch (B=128) recurrence; the
    large projection is sharded over the vocab dim (core c owns columns
    [c*1250, (c+1)*1250)), producing per (b,t) S_c = sum_v exp(logit_v).
    The label logit is a row-wise dot with a host-pregathered row of
    softmax_w^T, gated to core 0.  Host: xent = log(sum_c S_c) - LD.
  * All matmuls run fp8e4m3 + MatmulPerfMode.DoubleRow (K virtualized
    to 256 -> ~1.8x the bf16 streaming rate).  Scales: stationaries
    (x^T, o^T, h^T) carry SH=8x, weights SW=32x (with the sigmoid-trick
    0.5 folded into f/i/o weight columns), undone by ACT scale 1/256.
  * Host pre-gathers/transposes the embedded inputs (xt8) and the label
    rows of softmax_w^T (wl8) -- both are pure data layout, no FLOPs --
    so the loop has no indirect DMA and no PE transposes for x.
  * The h state lives ONLY in transposed (stationary) layout:
    hT_new = (1-m)*hT_old + oT, with (1-m) broadcast along partitions
    from a host-shipped slen row (slbc).  No h transposes.
  * Emission order keeps the PE fed with ready work (next step's x-part
    gates, the previous step's projection) while the serial cell chain
    runs on ACT/DVE, so the HAM clock gate stays at 2.4 GHz.

Assumes b0 = b1 = softmax_b = 0 (verified at runtime).
"""

import numpy as np
import ml_dtypes


def _ensure_path():
    try:
        import concourse  # noqa: F401
    except ImportError:
        import sys

        for p in ("/opt/trn_rl_repo", "/root/.axon_site/_ro/trn_rl_repo"):
            if p not in sys.path:
                sys.path.append(p)


_ensure_path()

from contextlib import ExitStack  # noqa: E402

import concourse.bass as bass  # noqa: E402
import concourse.bacc as bacc  # noqa: E402
import concourse.tile as tile  # noqa: E402
from concourse import mybir  # noqa: E402
from concourse.alu_op_type import AluOpType as OP  # noqa: E402
from concourse.bass_utils import run_bass_kernel_spmd  # noqa: E402
from concourse.masks import make_identity  # noqa: E402

dt = mybir.dt
AF = mybir.ActivationFunctionType
DR = mybir.MatmulPerfMode.DoubleRow

import os as _os

B = 128
T = int(_os.environ.get("KERNEL_T_OVERRIDE", "64"))
H = 512
V = 10000
NCORES = 8
VSH = V // NCORES  # 1250
G = 4 * H  # 2048
KH = H // 128  # 4 k-chunks per 512-wide contraction
MM_DT = dt.bfloat16
F8 = dt.float8e4
NP_F8 = ml_dtypes.float8_e4m3
NP_BF16 = ml_dtypes.bfloat16
SW = 32.0   # weight scale into fp8
SH = 8.0    # stationary-activation scale into fp8
GS = 1.0 / (SW * SH)  # activation un-scale
# projection free-dim chunks (PSUM bank = 512 fp32)
PCHUNKS = [(0, 512), (512, 1024), (1024, VSH)]

_CACHE: dict = {}


def _dr2(ap512, d):
    """[128, 512] T-layout slice for DoubleRow chunk d: [128, 2, 128]."""
    return ap512[:, 256 * d:256 * (d + 1)].rearrange("p (two m) -> p two m",
                                                     two=2)


def _emit(nc, tc, ext):
    f32 = dt.float32
    with ExitStack() as ctx:
        cpool = ctx.enter_context(tc.tile_pool(name="const", bufs=1))
        state = ctx.enter_context(tc.tile_pool(name="state", bufs=2))
        wp = ctx.enter_context(tc.tile_pool(name="work", bufs=3))
        gpsum = ctx.enter_context(tc.tile_pool(name="gpsum", bufs=2, space="PSUM"))
        tpsum = ctx.enter_context(tc.tile_pool(name="tpsum", bufs=1, space="PSUM"))
        ppsum = ctx.enter_context(tc.tile_pool(name="ppsum", bufs=1, space="PSUM"))

        # ---- constants / inputs -------------------------------------------
        ldsc = cpool.tile([B, 1], f32)
        nc.sync.dma_start(ldsc[:], ext["ldsc"][:, :])
        Mh = cpool.tile([B, T], f32)   # 0.5*m
        nc.sync.dma_start(Mh[:], ext["Mh"][:, :])
        M1h = cpool.tile([B, T], f32)  # 1-0.5*m
        nc.sync.dma_start(M1h[:], ext["M1h"][:, :])
        M4 = cpool.tile([B, T], f32)   # 4*m
        nc.sync.dma_start(M4[:], ext["M4"][:, :])
        slbc = cpool.tile([128, H], MM_DT)  # slen[b] bcast along p, 4x tiled
        nc.sync.dma_start(slbc[:], ext["slbc"][:, :])

        # per-k-chunk DMAs: first gate matmuls start before the full set lands
        w0 = cpool.tile([128, 2 * KH, G], F8)
        for k in range(2 * KH):
            nc.sync.dma_start(w0[:, k, :], ext["w0"][k, :, :])
        w1 = cpool.tile([128, 2 * KH, G], F8)
        for k in range(2 * KH):
            nc.sync.dma_start(w1[:, k, :], ext["w1"][k, :, :])
        wsm = cpool.tile([128, KH, VSH], F8)
        nc.sync.dma_start(wsm[:], ext["wsm"][:, :, :].rearrange("k p n -> p k n"))
        # pre-transposed, pre-scaled embedded inputs and label rows
        tch = min(8, T)
        xt_all = cpool.tile([128, T, KH, B], F8)
        for tt_ in range(0, T, tch):
            nc.sync.dma_start(xt_all[:, tt_:tt_ + tch, :, :],
                              ext["xt8"][:, tt_:tt_ + tch, :, :])
        wl_all = cpool.tile([128, T, H], F8)
        for tt_ in range(0, T, tch):
            nc.sync.dma_start(wl_all[:, tt_:tt_ + tch, :],
                              ext["wl8"][:, tt_:tt_ + tch, :])

        identB = cpool.tile([128, 128], MM_DT)
        make_identity(nc, identB[:])

        Sacc = cpool.tile([B, T], f32)
        LDacc = cpool.tile([B, T], f32)

        # ---- initial states ------------------------------------------------
        c_st = {}
        hT_st = {}
        for li in (0, 1):
            c_st[li] = state.tile([B, H], f32, name=f"c{li}", tag=f"c{li}")
            nc.vector.memset(c_st[li][:], 0.0)
            hT_st[li] = cpool.tile([128, H], F8, name=f"hT{li}")
            nc.vector.memset(hT_st[li][:], 0.0)

        def alloc_gates(li):
            # layer-0 pairs double-buffer across steps (x-part of t+1 runs
            # while t is consumed); layer-1 needs only one live pair.
            return [gpsum.tile([B, G // 2], f32, name=f"g{li}", tag=f"g{li}",
                               bufs=(2 if li == 0 else 1))
                    for _ in (0, 1)]

        def gates_part(halves, lhs_fn, w_tile, part, start, stop):
            # half-outer: each [B,1024] PSUM half finishes its accumulation
            # early so the cell's ACT read starts sooner.  DoubleRow: each
            # (half, n) slice accumulates K=512 as two K=256 chunks.
            k0 = 0 if part == "x" else KH
            for half in (0, 1):
                gh = halves[half]
                for d in (0, 1):
                    lhs = lhs_fn(d)
                    for n in (0, 1):
                        sl = slice(512 * n, 512 * (n + 1))
                        wsl = slice(1024 * half + 512 * n,
                                    1024 * half + 512 * (n + 1))
                        nc.tensor.matmul(gh[:, sl], lhs,
                                         w_tile[:, k0 + 2 * d:k0 + 2 * d + 2, wsl],
                                         start=(start and d == 0),
                                         stop=(stop and d == 1),
                                         perf_mode=DR)

        def cell(t, li, ghalves):
            """LSTM cell elementwise chain. gates ordered [f, i | o, cg].

            All gate columns need the same Tanh(g/256) (sigmoid 0.5 factors
            are folded into the f/i/o weight columns), so each PSUM half is
            one ACT op.  Returns o8 = 8*m*h_new (bf16, B-layout)."""
            gA, gB = ghalves
            mht = Mh[:, t:t + 1]
            m1ht = M1h[:, t:t + 1]
            m4t = M4[:, t:t + 1]

            # split the gate halves into 512-col ACT slices so the DVE chain
            # starts after the first slice, shortening the serial chain
            thA = wp.tile([B, G // 2], MM_DT, name="thA", tag="thA")
            nc.scalar.activation(thA[:, 0:512], gA[:, 0:512], AF.Tanh, scale=GS)
            fp = wp.tile([B, H], MM_DT, name="fp", tag="fp")
            nc.vector.tensor_scalar(out=fp[:], in0=thA[:, 0:512], scalar1=mht,
                                    scalar2=m1ht, op0=OP.mult, op1=OP.add)
            nc.scalar.activation(thA[:, 512:1024], gA[:, 512:1024], AF.Tanh,
                                 scale=GS)
            ip = wp.tile([B, H], MM_DT, name="ip", tag="ip")
            nc.vector.tensor_scalar(out=ip[:], in0=thA[:, 512:1024], scalar1=mht,
                                    scalar2=mht, op0=OP.mult, op1=OP.add)
            thB = wp.tile([B, G // 2], MM_DT, name="thB", tag="thB")
            nc.scalar.activation(thB[:, 512:1024], gB[:, 512:1024], AF.Tanh,
                                 scale=GS)
            r = wp.tile([B, H], f32, name="r", tag="r")
            nc.vector.tensor_tensor(out=r[:], in0=fp[:], in1=c_st[li][:],
                                    op=OP.mult)
            q = wp.tile([B, H], MM_DT, name="q", tag="q")
            nc.vector.tensor_tensor(out=q[:], in0=ip[:], in1=thB[:, 512:1024],
                                    op=OP.mult)
            nc.scalar.activation(thB[:, 0:512], gB[:, 0:512], AF.Tanh, scale=GS)
            osm = wp.tile([B, H], MM_DT, name="osm", tag="osm")
            nc.vector.tensor_scalar(out=osm[:], in0=thB[:, 0:512], scalar1=m4t,
                                    scalar2=m4t, op0=OP.mult, op1=OP.add)
            c_new = state.tile([B, H], f32, name=f"c{li}", tag=f"c{li}")
            o8 = wp.tile([B, H], MM_DT, name=f"o{li}", tag=f"o{li}")
            tc_ = wp.tile([B, H], MM_DT, name="tc_", tag="tc_")
            # halves: tanh(c) and o emerge 256 cols at a time -> transposes
            # and the layer-1 x gates start half a chain earlier
            for hf in (0, 1):
                sl = slice(256 * hf, 256 * (hf + 1))
                nc.vector.tensor_tensor(out=c_new[:, sl], in0=r[:, sl],
                                        in1=q[:, sl], op=OP.add)
                nc.scalar.activation(tc_[:, sl], c_new[:, sl], AF.Tanh)
                nc.vector.tensor_tensor(out=o8[:, sl], in0=osm[:, sl],
                                        in1=tc_[:, sl], op=OP.mult)
            c_st[li] = c_new
            return o8

        def transpose_o(o8, li):
            # transpose as a REGULAR matmul by identity (out = o8.T blockwise):
            # transpose-mode runs at the cold clock and does not count as
            # PE-busy for the HAM; a normal N=128 matmul is ~3x faster warm
            # and keeps the clock gate open.
            ps = tpsum.tile([128, H], f32, name="tp", tag="tp")
            for kc in range(KH):
                sl = slice(128 * kc, 128 * (kc + 1))
                nc.tensor.matmul(ps[:, sl], o8[:, sl], identB[:],
                                 start=True, stop=True)
            oT = wp.tile([128, H], F8, name=f"oT{li}", tag=f"oT{li}")
            nc.scalar.copy(oT[:, 0:256], ps[:, 0:256])
            nc.vector.tensor_copy(oT[:, 256:512], ps[:, 256:512])
            return oT

        def update_hT(li, oT, mbc):
            # active rows (m=1) take oT; frozen rows keep hT_old untouched --
            # one in-place predicated copy instead of mult+add
            nc.vector.copy_predicated(out=hT_st[li][:], mask=mbc[:],
                                      data=oT[:])

        def project(t, o1T8, o1_8):
            sps = []
            for (n0, n1) in PCHUNKS:
                w = n1 - n0
                pp = ppsum.tile([128, 512], f32, name="pp", tag="pp")
                for d in (0, 1):
                    nc.tensor.matmul(pp[:, 0:w], _dr2(o1T8[:], d),
                                     wsm[:, 2 * d:2 * d + 2, n0:n1],
                                     start=(d == 0), stop=(d == 1),
                                     perf_mode=DR)
                sp_i = wp.tile([B, 1], f32, name="sp_i", tag="sp_i")
                exp_scr = wp.tile([B, 512], MM_DT, name="exp_scr", tag="exp_scr")
                nc.scalar.activation(exp_scr[:, 0:w], pp[:, 0:w], AF.Exp,
                                     scale=GS, accum_out=sp_i[:])
                sps.append(sp_i)
            s01 = wp.tile([B, 1], f32, name="s01", tag="s01")
            nc.vector.tensor_tensor(out=s01[:], in0=sps[0][:], in1=sps[1][:],
                                    op=OP.add)
            nc.vector.tensor_tensor(out=Sacc[:, t:t + 1], in0=s01[:],
                                    in1=sps[2][:], op=OP.add)
            ld_scr = wp.tile([B, H], MM_DT, name="ld_scr", tag="ld_scr")
            nc.vector.scalar_tensor_tensor(out=ld_scr[:], in0=o1_8[:],
                                           scalar=GS, in1=wl_all[:, t, :],
                                           op0=OP.mult, op1=OP.mult,
                                           accum_out=LDacc[:, t:t + 1])

        # ---- software-pipelined main loop ---------------------------------
        # Emission order = per-engine issue order.  Keep ready MM work (next
        # step's x-part, previous step's projection) queued on the PE while
        # the serial cell chains run on ACT/DVE, so the PE never idles past
        # the HAM re-throttle window.
        def xt_lhs(t):
            return lambda d: xt_all[:, t, 2 * d:2 * d + 2, :]

        g0 = alloc_gates(0)
        gates_part(g0, xt_lhs(0), w0, "x", start=True, stop=True)  # t=0: no rec
        o1_prev = None
        o1T_prev = None
        for t in range(T):
            # (1-m[t]) broadcast along partitions, tiled 4x: for hT updates
            if t + 1 < T:
                m1bc = wp.tile([128, H], dt.uint8, name="m1bc", tag="m1bc")
                nc.vector.tensor_scalar(out=m1bc[:], in0=slbc[:],
                                        scalar1=float(t), scalar2=None,
                                        op0=OP.is_gt)
            g1 = None
            if t > 0:
                g1 = alloc_gates(1)
                gates_part(g1, lambda d: _dr2(hT_st[1][:], d), w1, "h",
                           start=True, stop=False)
            o0 = cell(t, 0, g0)
            if t + 1 < T:
                g0 = alloc_gates(0)
                gates_part(g0, xt_lhs(t + 1), w0, "x", start=True, stop=False)
            o0T = transpose_o(o0, 0)
            if g1 is None:
                g1 = alloc_gates(1)
                gates_part(g1, lambda d: _dr2(o0T[:], d), w1, "x",
                           start=True, stop=True)
            else:
                gates_part(g1, lambda d: _dr2(o0T[:], d), w1, "x",
                           start=False, stop=True)
            if t + 1 < T:
                update_hT(0, o0T, m1bc)
            if o1T_prev is not None:
                project(t - 1, o1T_prev, o1_prev)
            o1 = cell(t, 1, g1)
            if t + 1 < T:
                gates_part(g0, lambda d: _dr2(hT_st[0][:], d), w0, "h",
                           start=False, stop=True)
            o1T = transpose_o(o1, 1)
            if t + 1 < T:
                update_hT(1, o1T, m1bc)
            o1_prev, o1T_prev = o1, o1T
        project(T - 1, o1T_prev, o1_prev)

        # gate LD to core 0 (ldsc = 1 there, 0 elsewhere)
        nc.vector.tensor_scalar(out=LDacc[:], in0=LDacc[:],
                                scalar1=ldsc[:, 0:1], scalar2=None,
                                op0=OP.mult)
        nc.sync.dma_start(ext["S"][:, :], Sacc[:])
        nc.sync.dma_start(ext["LD"][:, :], LDacc[:])


def _build():
    if "nc" in _CACHE:
        return _CACHE["nc"]
    nc = bacc.Bacc("TRN2", target_bir_lowering=False, debug=False,
                   num_devices=NCORES)
    ext = {
        "ldsc": nc.declare_dram_parameter("ldsc", [B, 1], dt.float32,
                                          isOutput=False),
        "Mh": nc.declare_dram_parameter("Mh", [B, T], dt.float32,
                                        isOutput=False),
        "M1h": nc.declare_dram_parameter("M1h", [B, T], dt.float32,
                                         isOutput=False),
        "M4": nc.declare_dram_parameter("M4", [B, T], dt.float32,
                                        isOutput=False),
        "slbc": nc.declare_dram_parameter("slbc", [128, H], MM_DT,
                                          isOutput=False),
        "w0": nc.declare_dram_parameter("w0", [2 * KH, 128, G], F8,
                                        isOutput=False),
        "w1": nc.declare_dram_parameter("w1", [2 * KH, 128, G], F8,
                                        isOutput=False),
        "wsm": nc.declare_dram_parameter("wsm", [KH, 128, VSH], F8,
                                         isOutput=False),
        "xt8": nc.declare_dram_parameter("xt8", [128, T, KH, B], F8,
                                         isOutput=False),
        "wl8": nc.declare_dram_parameter("wl8", [128, T, H], F8,
                                         isOutput=False),
        "S": nc.declare_dram_parameter("S", [B, T], dt.float32, isOutput=True),
        "LD": nc.declare_dram_parameter("LD", [B, T], dt.float32, isOutput=True),
    }
    with tile.TileContext(nc) as tc:
        _emit(nc, tc, ext)
    nc.compile()
    _CACHE["nc"] = nc
    return nc


def _pack_w(Wx, Wh):
    w = np.concatenate([np.asarray(Wx, np.float32), np.asarray(Wh, np.float32)],
                       axis=0)  # [2H, 4H] rows: x-part then h-part
    # reorder gate blocks [i, cg, f, o] -> [f, i, o, cg]
    w = np.concatenate([w[:, 1024:1536], w[:, 0:512], w[:, 1536:2048],
                        w[:, 512:1024]], axis=1)
    # sigmoid trick: tanh(0.5*g) for f/i/o -> fold 0.5 into those columns
    w[:, 0:1536] *= 0.5
    w *= SW
    return np.ascontiguousarray(w.reshape(2 * KH, 128, G)).astype(NP_F8)


def kernel(features, labels, seq_lengths, seq_mask, embedding,
           W0x, W0h, b0, W1x, W1h, b1, softmax_w, softmax_b,
           _trace_dir=None):
    for name, b in (("b0", b0), ("b1", b1), ("softmax_b", softmax_b)):
        if np.any(np.asarray(b, np.float32) != 0.0):
            raise NotImplementedError(f"{name} != 0 not supported")

    feats = np.asarray(features, np.int64)[:, :T]
    labs = np.asarray(labels, np.int64)[:, :T]
    slen = np.asarray(seq_lengths, np.int32).astype(np.float32)  # [B]
    mask = np.asarray(seq_mask, np.float32)[:, :T]
    emb = np.asarray(embedding, np.float32)
    w0 = _pack_w(W0x, W0h)
    w1 = _pack_w(W1x, W1h)
    wsm_f = np.asarray(softmax_w, np.float32) * SW
    wsm_r = wsm_f.reshape(KH, 128, V)

    # host-side data prep (layout only, no model math):
    # xt8[p, t, kc, b] = SH * emb[features[b, t], kc*128 + p]
    x = emb[feats] * SH                              # [B, T, H]
    xt8 = np.ascontiguousarray(
        x.reshape(B, T, KH, 128).transpose(3, 1, 2, 0)).astype(NP_F8)
    # wl8[b, t, :] = SW * softmax_w[:, labels[b, t]]
    wl8 = np.ascontiguousarray(
        wsm_f.T[labs].transpose(0, 1, 2)).astype(NP_F8)  # [B, T, H]
    # masks
    m = (np.arange(T)[None, :] < slen[:, None]).astype(np.float32)  # [B, T]
    Mh = 0.5 * m
    M1h = 1.0 - 0.5 * m
    M4 = 4.0 * m
    # slbc[p, kc*128+b] = slen[b]
    slbc = np.ascontiguousarray(
        np.tile(slen[None, None, :], (128, KH, 1)).reshape(128, H)
    ).astype(NP_BF16)

    nc = _build()
    in_maps = []
    for c in range(NCORES):
        in_maps.append({
            "ldsc": np.full((B, 1), 1.0 if c == 0 else 0.0, np.float32),
            "Mh": Mh, "M1h": M1h, "M4": M4, "slbc": slbc,
            "w0": w0,
            "w1": w1,
            "wsm": np.ascontiguousarray(
                wsm_r[:, :, c * VSH:(c + 1) * VSH]).astype(NP_F8),
            "xt8": xt8,
            "wl8": wl8,
        })

    kwargs = {}
    if _trace_dir is not None:
        kwargs = dict(trace=True, tmpdir=_trace_dir)
    res = run_bass_kernel_spmd(nc, in_maps, list(range(NCORES)), **kwargs)
    _CACHE["last_results"] = res

    S = np.zeros((B, T), np.float64)
    LD = np.zeros((B, T), np.float64)
    for c in range(NCORES):
        S += np.asarray(res.results[c]["S"], np.float64)
        LD += np.asarray(res.results[c]["LD"], np.float64)

    xent = np.log(S) - LD
    loss_t = (xent * mask).sum(axis=0) / (mask.sum(axis=0) + 1e-12)
    cost = loss_t.mean()
    return np.asarray(cost, np.float32)
